# revision 7
# baseline (speedup 1.0000x reference)
"""DifferentiableHungarianLoss kernel for 8 TRN2 NeuronCores.

reference semantics:
    A = latent[0], B = latent[1]                       # [512, 512] each
    cost[i, j] = ||A_i - B_j||_2                       # [512, 512] cdist
    P = Hungarian(cost)  (exact LAP, via host callback in the reference too)
    loss = |sum(P * cost) - trace(cost)| / 512
    returns (loss, arange(512), argmax(P, axis=1))

Device: the cost matrix (all the tensor math) is computed on the 8 cores with
a 4x2 2D block sharding -- core k owns cost block [128 A-rows x 256 B-rows],
so each core only DMAs 768KB (A quarter 256KB + B half 512KB) instead of a
replicated 1.125MB.  Shards are marshalled in transposed (column-major)
layout so the TensorEngine needs no on-chip transposes: PSUM accumulates
    psum = A.B^T - an2/2 (x) 1 - 1 (x) bn2/2
(four K=128 matmuls plus two rank-1 matmuls; the norms come from DVE squares
+ GpSimd cross-partition reduces), and ACT evicts cost = sqrt(-2 * psum).

Host: the Jonker-Volgenant solve (inherently sequential; the reference runs it
through jax.pure_callback on host as well -- it cannot lower to neuron) plus
the final scalar arithmetic.
"""

import numpy as np

N = 512
D = 512
M_BLK = 128  # A rows per core  (4-way split)
N_BLK = 256  # B rows per core  (2-way split)
N_CORES = 8

_compiled = None


def _build():
    import concourse.bass as bass
    import concourse.tile as tile
    from concourse import bacc, mybir
    from contextlib import ExitStack

    f32 = mybir.dt.float32
    P = 128

    nc = bacc.Bacc("TRN2", target_bir_lowering=False, debug=False,
                   num_devices=N_CORES)

    # transposed shards: at[d, m], bt[d, n]
    at_dram = nc.dram_tensor("at", [D, M_BLK], f32, kind="ExternalInput").ap()
    bt_dram = nc.dram_tensor("bt", [D, N_BLK], f32, kind="ExternalInput").ap()
    cost_dram = nc.dram_tensor("cost", [M_BLK, N_BLK], f32,
                               kind="ExternalOutput").ap()

    with tile.TileContext(nc) as tc, ExitStack() as ctx:
        import concourse.bass_isa as bass_isa

        in_pool = ctx.enter_context(tc.tile_pool(name="inp", bufs=1))
        tp_pool = ctx.enter_context(tc.tile_pool(name="tp", bufs=1))
        out_pool = ctx.enter_context(tc.tile_pool(name="out", bufs=1))
        ps_acc = ctx.enter_context(tc.tile_pool(name="psacc", bufs=1, space="PSUM"))

        # [p, c, x]: element (c*128+p, x) of the dram shard
        at_t = in_pool.tile([P, 4, M_BLK], f32)
        bt_t = in_pool.tile([P, 4, N_BLK], f32)
        nc.sync.dma_start(at_t[:], at_dram.rearrange("(c p) m -> p c m", p=P))
        for h in range(2):
            nc.sync.dma_start(
                bt_t[:, h * 2:(h + 1) * 2, :],
                bt_dram.rearrange("(c p) n -> p c n", p=P)[:, h * 2:(h + 1) * 2, :])

        # norms: squares on DVE, cross-partition sum on GpSimd, chunk-sum on DVE
        atsq = tp_pool.tile([P, 4, M_BLK], f32)
        btsq = tp_pool.tile([P, 4, N_BLK], f32)
        nc.vector.tensor_mul(atsq[:], at_t[:], at_t[:])
        nc.vector.tensor_mul(btsq[:], bt_t[:], bt_t[:])
        an2p = tp_pool.tile([P, 4, M_BLK], f32)
        bn2p = tp_pool.tile([P, 4, N_BLK], f32)
        nc.gpsimd.partition_all_reduce(an2p[:], atsq[:], channels=P,
                                       reduce_op=bass_isa.ReduceOp.add)
        nc.gpsimd.partition_all_reduce(bn2p[:], btsq[:], channels=P,
                                       reduce_op=bass_isa.ReduceOp.add)
        an2T = tp_pool.tile([1, M_BLK], f32)
        bn2T = tp_pool.tile([1, N_BLK], f32)
        nc.vector.tensor_reduce(an2T[:], an2p[0:1].rearrange("o c m -> o m c"),
                                mybir.AxisListType.X, mybir.AluOpType.add)
        nc.vector.tensor_reduce(bn2T[:], bn2p[0:1].rearrange("o c n -> o n c"),
                                mybir.AxisListType.X, mybir.AluOpType.add)
        negh_m = tp_pool.tile([1, M_BLK], f32)
        negh_n = tp_pool.tile([1, N_BLK], f32)
        nc.gpsimd.memset(negh_m[:], -0.5)
        nc.gpsimd.memset(negh_n[:], -0.5)

        # psum = A.B^T - an2/2 - bn2/2   (per-chunk matmuls + two rank-1s)
        d2_ps = ps_acc.tile([P, N_BLK], f32)
        for c in range(4):
            nc.tensor.matmul(d2_ps[:], at_t[:, c, :], bt_t[:, c, :],
                             start=(c == 0), stop=False)
        nc.tensor.matmul(d2_ps[:], an2T[:], negh_n[:], start=False, stop=False)
        nc.tensor.matmul(d2_ps[:], negh_m[:], bn2T[:], start=False, stop=True)

        # cost = sqrt(-2 * psum)
        out_t = out_pool.tile([P, N_BLK], f32)
        nc.scalar.activation(out_t[:], d2_ps[:],
                             mybir.ActivationFunctionType.Sqrt, scale=-2.0)
        nc.sync.dma_start(cost_dram[:], out_t[:])

    nc.compile()
    return nc


def _get_compiled():
    global _compiled
    if _compiled is None:
        _compiled = _build()
    return _compiled


def _run_device_cost(lat, trace=False):
    """Run the 8-core cost-matrix kernel; returns (cost [512,512] f32, results)."""
    from concourse.bass_utils import run_bass_kernel_spmd

    nc = _get_compiled()
    AT = np.ascontiguousarray(np.asarray(lat[0], dtype=np.float32).T)
    BT = np.ascontiguousarray(np.asarray(lat[1], dtype=np.float32).T)
    in_maps = []
    for k in range(N_CORES):
        mi, nj = k // 2, k % 2
        in_maps.append({
            "at": np.ascontiguousarray(AT[:, mi * M_BLK:(mi + 1) * M_BLK]),
            "bt": np.ascontiguousarray(BT[:, nj * N_BLK:(nj + 1) * N_BLK]),
        })
    res = run_bass_kernel_spmd(nc, in_maps, list(range(N_CORES)), trace=trace)
    cost = np.empty((N, N), dtype=np.float32)
    for k in range(N_CORES):
        mi, nj = k // 2, k % 2
        cost[mi * M_BLK:(mi + 1) * M_BLK,
             nj * N_BLK:(nj + 1) * N_BLK] = res.results[k]["cost"]
    return cost, res


def _lap_jv(cost):
    """Exact Jonker-Volgenant LAP (dual potentials + shortest augmenting
    path); identical algorithm to the reference / scipy."""
    cost = np.asarray(cost, dtype=np.float64)
    n = cost.shape[0]
    INF = np.inf
    u = np.zeros(n + 1)
    v = np.zeros(n + 1)
    p = np.zeros(n + 1, dtype=np.int64)
    way = np.zeros(n + 1, dtype=np.int64)
    for i in range(1, n + 1):
        p[0] = i
        j0 = 0
        minv = np.full(n + 1, INF)
        used = np.zeros(n + 1, dtype=bool)
        while True:
            used[j0] = True
            i0 = p[j0]
            cur = cost[i0 - 1, :] - u[i0] - v[1:]
            free = ~used[1:]
            upd = free & (cur < minv[1:])
            minv[1:][upd] = cur[upd]
            way[1:][upd] = j0
            m = np.where(free, minv[1:], INF)
            j1 = int(np.argmin(m)) + 1
            delta = m[j1 - 1]
            iu = np.nonzero(used)[0]
            u[p[iu]] += delta
            v[iu] -= delta
            minv[1:][free] -= delta
            j0 = j1
            if p[j0] == 0:
                break
        while j0 != 0:
            j1 = way[j0]
            p[j0] = p[j1]
            j0 = j1
    col_of_row = np.empty(n, dtype=np.int64)
    col_of_row[p[1:] - 1] = np.arange(n)
    return col_of_row


def _solve_lap(cost):
    try:
        from scipy.optimize import linear_sum_assignment
        _, col = linear_sum_assignment(np.asarray(cost, dtype=np.float64))
        return col
    except Exception:
        return _lap_jv(cost)


def kernel(latent):
    lat = np.asarray(latent)
    cost, _ = _run_device_cost(lat)
    c64 = cost.astype(np.float64)
    col_ind = _solve_lap(c64)
    predicted = c64[np.arange(N), col_ind].sum()
    ideal = np.trace(c64)
    loss = np.float32(abs(predicted - ideal) / N)
    row_ind = np.arange(N, dtype=np.int32)
    return loss, row_ind, col_ind.astype(np.int32)


# revision 8
# speedup vs baseline: 1.7685x; 1.7685x over previous
"""DifferentiableHungarianLoss kernel for 8 TRN2 NeuronCores.

reference semantics:
    A = latent[0], B = latent[1]                       # [512, 512] each
    cost[i, j] = ||A_i - B_j||_2                       # [512, 512] cdist
    P = Hungarian(cost)  (exact LAP, via host callback in the reference too)
    loss = |sum(P * cost) - trace(cost)| / 512
    returns (loss, arange(512), argmax(P, axis=1))

Device: the cost matrix (all the tensor math) is computed on the 8 cores with
a 4x2 2D block sharding -- core k owns cost block [128 A-rows x 256 B-rows],
so each core only DMAs 768KB (A quarter 256KB + B half 512KB) instead of a
replicated 1.125MB.  Shards are marshalled in transposed (column-major)
layout so the TensorEngine needs no on-chip transposes: PSUM accumulates
    psum = A.B^T - an2/2 (x) 1 - 1 (x) bn2/2
(four K=128 matmuls plus two rank-1 matmuls; the norms come from DVE squares
+ GpSimd cross-partition reduces), and ACT evicts cost = sqrt(-2 * psum).

Host: the Jonker-Volgenant solve (inherently sequential; the reference runs it
through jax.pure_callback on host as well -- it cannot lower to neuron) plus
the final scalar arithmetic.
"""

import numpy as np

N = 512
D = 512
M_BLK = 128  # A rows per core  (4-way split)
N_BLK = 256  # B rows per core  (2-way split)
N_CORES = 8

_compiled = None


def _build():
    import concourse.bass as bass
    import concourse.tile as tile
    from concourse import bacc, mybir
    from contextlib import ExitStack

    f32 = mybir.dt.float32
    P = 128

    nc = bacc.Bacc("TRN2", target_bir_lowering=False, debug=False,
                   num_devices=N_CORES)

    # transposed shards: at[d, m], bt[d, n]
    at_dram = nc.dram_tensor("at", [D, M_BLK], f32, kind="ExternalInput").ap()
    bt_dram = nc.dram_tensor("bt", [D, N_BLK], f32, kind="ExternalInput").ap()
    cost_dram = nc.dram_tensor("cost", [M_BLK, N_BLK], f32,
                               kind="ExternalOutput").ap()

    with tile.TileContext(nc) as tc, ExitStack() as ctx:
        import concourse.bass_isa as bass_isa

        in_pool = ctx.enter_context(tc.tile_pool(name="inp", bufs=1))
        tp_pool = ctx.enter_context(tc.tile_pool(name="tp", bufs=1))
        out_pool = ctx.enter_context(tc.tile_pool(name="out", bufs=1))
        ps_acc = ctx.enter_context(tc.tile_pool(name="psacc", bufs=1, space="PSUM"))

        # [p, c, x]: element (c*128+p, x) of the dram shard
        at_t = in_pool.tile([P, 4, M_BLK], f32)
        bt_t = in_pool.tile([P, 4, N_BLK], f32)
        nc.sync.dma_start(at_t[:], at_dram.rearrange("(c p) m -> p c m", p=P))
        for h in range(2):
            nc.sync.dma_start(
                bt_t[:, h * 2:(h + 1) * 2, :],
                bt_dram.rearrange("(c p) n -> p c n", p=P)[:, h * 2:(h + 1) * 2, :])

        # norms: squares + chunk-sums on DVE -> [128, X] per-partition partials
        # whose cross-partition sum is folded into the PSUM accumulation via
        # one matmul against an all--0.5 tile per side:
        #   matmul(lhsT=atsq4, rhs=-.5ones)[m,n] = -0.5 * an2[m]
        #   matmul(lhsT=-.5ones, rhs=btsq4)[m,n] = -0.5 * bn2[n]
        atsq = tp_pool.tile([P, 4, M_BLK], f32)
        btsq = tp_pool.tile([P, 4, N_BLK], f32)
        nc.vector.tensor_mul(atsq[:], at_t[:], at_t[:])
        for h in range(2):
            nc.vector.tensor_mul(btsq[:, h * 2:(h + 1) * 2, :],
                                 bt_t[:, h * 2:(h + 1) * 2, :],
                                 bt_t[:, h * 2:(h + 1) * 2, :])
        atsq4 = tp_pool.tile([P, M_BLK], f32)
        btsq4 = tp_pool.tile([P, N_BLK], f32)
        nc.vector.tensor_reduce(atsq4[:], atsq[:].rearrange("p c m -> p m c"),
                                mybir.AxisListType.X, mybir.AluOpType.add)
        nc.vector.tensor_reduce(btsq4[:], btsq[:].rearrange("p c n -> p n c"),
                                mybir.AxisListType.X, mybir.AluOpType.add)
        negh = tp_pool.tile([P, N_BLK], f32)
        nc.gpsimd.memset(negh[:], -0.5)

        # psum = A.B^T - an2/2 - bn2/2
        d2_ps = ps_acc.tile([P, N_BLK], f32)
        nc.tensor.matmul(d2_ps[:], atsq4[:], negh[:], start=True, stop=False)
        for c in range(4):
            nc.tensor.matmul(d2_ps[:], at_t[:, c, :], bt_t[:, c, :],
                             start=False, stop=False)
        nc.tensor.matmul(d2_ps[:], negh[:, :M_BLK], btsq4[:],
                         start=False, stop=True)

        # cost = sqrt(-2 * psum)
        out_t = out_pool.tile([P, N_BLK], f32)
        nc.scalar.activation(out_t[:], d2_ps[:],
                             mybir.ActivationFunctionType.Sqrt, scale=-2.0)
        nc.sync.dma_start(cost_dram[:], out_t[:])

    nc.compile()
    return nc


def _get_compiled():
    global _compiled
    if _compiled is None:
        _compiled = _build()
    return _compiled


def _run_device_cost(lat, trace=False):
    """Run the 8-core cost-matrix kernel; returns (cost [512,512] f32, results)."""
    from concourse.bass_utils import run_bass_kernel_spmd

    nc = _get_compiled()
    AT = np.ascontiguousarray(np.asarray(lat[0], dtype=np.float32).T)
    BT = np.ascontiguousarray(np.asarray(lat[1], dtype=np.float32).T)
    in_maps = []
    for k in range(N_CORES):
        mi, nj = k // 2, k % 2
        in_maps.append({
            "at": np.ascontiguousarray(AT[:, mi * M_BLK:(mi + 1) * M_BLK]),
            "bt": np.ascontiguousarray(BT[:, nj * N_BLK:(nj + 1) * N_BLK]),
        })
    res = run_bass_kernel_spmd(nc, in_maps, list(range(N_CORES)), trace=trace)
    cost = np.empty((N, N), dtype=np.float32)
    for k in range(N_CORES):
        mi, nj = k // 2, k % 2
        cost[mi * M_BLK:(mi + 1) * M_BLK,
             nj * N_BLK:(nj + 1) * N_BLK] = res.results[k]["cost"]
    return cost, res


def _lap_jv(cost):
    """Exact Jonker-Volgenant LAP (dual potentials + shortest augmenting
    path); identical algorithm to the reference / scipy."""
    cost = np.asarray(cost, dtype=np.float64)
    n = cost.shape[0]
    INF = np.inf
    u = np.zeros(n + 1)
    v = np.zeros(n + 1)
    p = np.zeros(n + 1, dtype=np.int64)
    way = np.zeros(n + 1, dtype=np.int64)
    for i in range(1, n + 1):
        p[0] = i
        j0 = 0
        minv = np.full(n + 1, INF)
        used = np.zeros(n + 1, dtype=bool)
        while True:
            used[j0] = True
            i0 = p[j0]
            cur = cost[i0 - 1, :] - u[i0] - v[1:]
            free = ~used[1:]
            upd = free & (cur < minv[1:])
            minv[1:][upd] = cur[upd]
            way[1:][upd] = j0
            m = np.where(free, minv[1:], INF)
            j1 = int(np.argmin(m)) + 1
            delta = m[j1 - 1]
            iu = np.nonzero(used)[0]
            u[p[iu]] += delta
            v[iu] -= delta
            minv[1:][free] -= delta
            j0 = j1
            if p[j0] == 0:
                break
        while j0 != 0:
            j1 = way[j0]
            p[j0] = p[j1]
            j0 = j1
    col_of_row = np.empty(n, dtype=np.int64)
    col_of_row[p[1:] - 1] = np.arange(n)
    return col_of_row


def _solve_lap(cost):
    try:
        from scipy.optimize import linear_sum_assignment
        _, col = linear_sum_assignment(np.asarray(cost, dtype=np.float64))
        return col
    except Exception:
        return _lap_jv(cost)


def kernel(latent):
    lat = np.asarray(latent)
    cost, _ = _run_device_cost(lat)
    c64 = cost.astype(np.float64)
    col_ind = _solve_lap(c64)
    predicted = c64[np.arange(N), col_ind].sum()
    ideal = np.trace(c64)
    loss = np.float32(abs(predicted - ideal) / N)
    row_ind = np.arange(N, dtype=np.int32)
    return loss, row_ind, col_ind.astype(np.int32)
